# revision 18
# baseline (speedup 1.0000x reference)
"""Trainium2 Bass kernel: causal multi-head attention with RoPE.

Model: B=4, L=2048, H=2048, NH=16 heads, head_dim=128.
  q = x @ Wq.T ; k = x @ Wk.T ; v = x @ Wv.T        (per-head split)
  q, k <- RoPE(q, k)
  attn = softmax(mask(q k^T / sqrt(hd)))
  out  = (attn @ v) heads-concat @ Wo.T

Sharding (8 cores): hybrid batch x tensor-parallel.  Core c handles
batch b = c//2 and heads half*8..half*8+7 with half = c%2.  Wq/Wk/Wv are
column-sharded (8 heads per core), Wo row-sharded; each core produces a
partial y[b] and the host sums the two partials per batch (the unshard
step) and concatenates batches.

Per-core dataflow (SBUF-resident, bf16 inputs / fp32 accumulation):
  phase A: merged Q^T/K^T projection per 512-pos x chunk (x loaded once
           for both) with fused RoPE; startup DMA split in small pieces
           across both HWDGE rings; gpsimd ucode lib pre-warmed.
  phase B: flash-style causal attention per (head, 512-q-chunk PAIR):
           the two chunks share one 2-bank PSUM S^T tile so ONE exp
           ACTIVATE covers both (the 352-cycle ACT fixed cost was
           co-critical with the PE); softmax denominators accumulate on
           DVE in fp16 and partition-reduce on gpsimd (partition_all_
           reduce) -- no PE ones-matmuls.  V pieces 4-7 and the Wo
           chunks 0/1 are emitted as PE filler inside the ACT-bound
           attention stretches.
  phase C: remaining output projection, y^T -> DRAM fp32.
"""

import math
import numpy as np

B, L, H, NH, HD = 4, 2048, 2048, 16, 128
ROPE_BASE = 10000.0
NCORES = 8
HPC = 8          # heads per core
QC = 512         # q chunk width
NQC = L // QC    # 4 q chunks
NKB = L // 128   # 16 kp blocks
VC = 256         # V-phase x piece width (pos)
SCALE = 1.0 / math.sqrt(HD)

_cache = {}


def _analyze_mask(mask2d):
    """Classify each (q_block, kp_block) 128x128 block of the [L, L] mask.

    Returns (block_kind[16][16] with 0=empty,1=full,2=mixed, patterns,
    pattern_idx dict keyed by block coords). mask2d is int32 [L, L],
    rows=q, cols=kp.
    """
    nb = L // 128
    kind = [[0] * nb for _ in range(nb)]
    patterns = []
    pat_key_to_idx = {}
    block_pat = {}
    for qb in range(nb):
        rows = mask2d[qb * 128:(qb + 1) * 128]
        for kb in range(nb):
            blk = rows[:, kb * 128:(kb + 1) * 128]
            s = int(blk.sum())
            if s == 0:
                kind[qb][kb] = 0
            elif s == 128 * 128:
                kind[qb][kb] = 1
            else:
                kind[qb][kb] = 2
                key = blk.tobytes()
                idx = pat_key_to_idx.get(key)
                if idx is None:
                    idx = len(patterns)
                    pat_key_to_idx[key] = idx
                    # stored transposed: S^T tiles are [kp, q]
                    patterns.append(np.ascontiguousarray(blk.T))
                block_pat[(qb, kb)] = idx
    return kind, patterns, block_pat


def _build(kind, block_pat, n_patterns):
    """Build the SPMD bass program (same for all 8 cores)."""
    import concourse.bass as bass
    import concourse.bass_isa as bass_isa
    import concourse.bacc as bacc
    import concourse.mybir as mybir
    import concourse.tile as tile

    fp32 = mybir.dt.float32
    bf16 = mybir.dt.bfloat16
    EXP = mybir.ActivationFunctionType.Exp

    nc = bacc.Bacc("TRN2", target_bir_lowering=False, debug=False)

    xT = nc.dram_tensor("xT", [H, L], bf16, kind="ExternalInput")
    wqT = nc.dram_tensor("wqT", [H, HPC * HD], bf16, kind="ExternalInput")
    wkT = nc.dram_tensor("wkT", [H, HPC * HD], bf16, kind="ExternalInput")
    wvT = nc.dram_tensor("wvT", [H, HPC * HD], bf16, kind="ExternalInput")
    woT = nc.dram_tensor("woT", [HPC * HD, H], bf16, kind="ExternalInput")
    cosd = nc.dram_tensor("cosd", [HD, L], bf16, kind="ExternalInput")
    sinmd = nc.dram_tensor("sinmd", [HD, L], bf16, kind="ExternalInput")
    npat = max(n_patterns, 1)
    maskd = nc.dram_tensor("maskd", [npat, 128, 128], bf16, kind="ExternalInput")
    yT = nc.dram_tensor("yT", [H, L], fp32, kind="ExternalOutput")

    NHC = H // 128  # 16 input-feature blocks

    with tile.TileContext(nc) as tc:
        with tc.tile_pool(name="persist", bufs=1, side="left") as persist:
            cst = persist.tile([128, npat * 128 + 128], bf16, tag="cst")
            warm = persist.tile([128, 8], fp32, tag="warm")
            QTa = persist.tile([HD, HPC, L], bf16, tag="qta")
            KTa = persist.tile([HD, HPC, L], bf16, tag="kta")

            # pre-load the gpsimd "attn" ucode library during startup DMA
            # (the first custom op otherwise pays a ~10us LOAD_LIB mid-kernel)
            nc.vector.memset(warm[:], 1.0)
            nc.gpsimd.partition_broadcast(warm[:, 4:8], warm[0:1, 0:4])

            # ---------------- phase A: Q/K projections + RoPE -------------
            wqk_cm = tc.tile_pool(name="wqk", bufs=1, side="right")
            wqk = wqk_cm.__enter__()
            ropec_cm = tc.tile_pool(name="ropec", bufs=1, side="right")
            ropec = ropec_cm.__enter__()
            xqk_cm = tc.tile_pool(name="xqk", bufs=3, side="right")
            xqk = xqk_cm.__enter__()
            tpool_cm = tc.tile_pool(name="tpool", bufs=2, side="right")
            tpool = tpool_cm.__enter__()
            psp_cm = tc.tile_pool(name="ps_proj", bufs=3, space="PSUM")
            psp = psp_cm.__enter__()

            wq_sb = wqk.tile([128, NHC, HPC * HD], bf16, tag="wq")
            wk_sb = wqk.tile([128, NHC, HPC * HD], bf16, tag="wk")
            cos_sb = ropec.tile([HD, L], bf16, tag="cos")
            sinm_sb = ropec.tile([HD, L], bf16, tag="sinm")

            wr_q = wqT[:].rearrange("(a p) m -> p a m", p=128)
            wr_k = wkT[:].rearrange("(a p) m -> p a m", p=128)

            def x_tile_dma(x_sb, js):
                xr = xT[:, js].rearrange("(a p) m -> p a m", p=128)
                for g in range(8):
                    nc.sync.dma_start(out=x_sb[:, 2 * g:2 * g + 2, :],
                                      in_=xr[:, 2 * g:2 * g + 2, :])

            # startup: interleave wq (sync ring) with x0 (act ring) in
            # small pieces so the first matmuls can start early
            x0_sb = xqk.tile([128, NHC, QC], bf16, tag="x", name="x0")
            xr0 = xT[:, 0:QC].rearrange("(a p) m -> p a m", p=128)
            for g in range(8):
                nc.sync.dma_start(out=wq_sb[:, 2 * g:2 * g + 2, :],
                                  in_=wr_q[:, 2 * g:2 * g + 2, :])
                nc.scalar.dma_start(out=x0_sb[:, 2 * g:2 * g + 2, :],
                                    in_=xr0[:, 2 * g:2 * g + 2, :])
            for g in range(8):
                nc.sync.dma_start(out=wk_sb[:, 2 * g:2 * g + 2, :],
                                  in_=wr_k[:, 2 * g:2 * g + 2, :])
            nc.scalar.dma_start(out=cos_sb[:], in_=cosd[:])
            nc.scalar.dma_start(out=sinm_sb[:], in_=sinmd[:])
            for p in range(n_patterns):
                nc.scalar.dma_start(out=cst[:, p * 128:(p + 1) * 128],
                                    in_=maskd[p])
            ones_col = npat * 128
            nc.vector.memset(cst[:, ones_col:ones_col + 128], 1.0)
            ones_sb = cst[:, ones_col:ones_col + 1]

            for j in range(NQC):
                js = slice(j * QC, (j + 1) * QC)
                if j == 0:
                    x_sb = x0_sb
                else:
                    x_sb = xqk.tile([128, NHC, QC], bf16, tag="x",
                                    name=f"x{j}")
                    x_tile_dma(x_sb, js)
                for w_sb, out_a, wtag in ((wq_sb, QTa, "q"), (wk_sb, KTa, "k")):
                    for h in range(HPC):
                        ps = psp.tile([128, QC], fp32, tag="ps_proj")
                        for hc in range(NHC):
                            nc.tensor.matmul(
                                ps[:],
                                w_sb[:, hc, h * HD:(h + 1) * HD],
                                x_sb[:, hc, :],
                                start=(hc == 0), stop=(hc == NHC - 1))
                        q = out_a[:, h, js]
                        nc.scalar.copy(q, ps[:])
                        # rotate-half: pure partition swap via SBUF->SBUF DMA
                        # (scalar ring: keeps the sync ring a pure prefetch
                        # stream with no compute-dependent head-of-line waits)
                        rq = tpool.tile([128, QC], bf16, tag="rot")
                        nc.scalar.dma_start(out=rq[0:64, :],
                                            in_=out_a[64:128, h, js])
                        nc.scalar.dma_start(out=rq[64:128, :],
                                            in_=out_a[0:64, h, js])
                        nc.vector.tensor_mul(rq[:], rq[:], sinm_sb[:, js])
                        nc.vector.tensor_mul(q, q, cos_sb[:, js])
                        nc.vector.tensor_add(q, q, rq[:])

            tpool_cm.__exit__(None, None, None)
            xqk_cm.__exit__(None, None, None)
            ropec_cm.__exit__(None, None, None)
            wqk_cm.__exit__(None, None, None)
            psp_cm.__exit__(None, None, None)

            # ---------------- V projection + attention + output -----------
            vp_cm = tc.tile_pool(name="vp", bufs=1, side="left")
            vp = vp_cm.__enter__()
            Va = vp.tile([128, NKB, HPC * HD], bf16, tag="va")
            otp_cm = tc.tile_pool(name="otp", bufs=1, side="left")
            otp = otp_cm.__enter__()
            OTa = otp.tile([HD, HPC, L], bf16, tag="ota")

            ppool_cm = tc.tile_pool(name="pp", bufs=3, side="right")
            ppool = ppool_cm.__enter__()
            accp_cm = tc.tile_pool(name="acc", bufs=2, side="right")
            accp = accp_cm.__enter__()
            redp_cm = tc.tile_pool(name="red", bufs=1, side="right")
            redp = redp_cm.__enter__()
            wvp_cm = tc.tile_pool(name="wvp", bufs=1, side="right")
            wvp = wvp_cm.__enter__()
            xvp_cm = tc.tile_pool(name="xvp", bufs=3, side="right")
            xvp = xvp_cm.__enter__()

            ps_s_cm = tc.tile_pool(name="ps_s", bufs=2, space="PSUM")
            ps_s = ps_s_cm.__enter__()
            ps_o_cm = tc.tile_pool(name="ps_o", bufs=1, space="PSUM")
            ps_o = ps_o_cm.__enter__()
            aux_cm = tc.tile_pool(name="ps_aux", bufs=2, space="PSUM")
            aux = aux_cm.__enter__()

            wv_sb = wvp.tile([128, NHC, HPC * HD], bf16, tag="wv")
            wr_v = wvT[:].rearrange("(a p) m -> p a m", p=128)
            for g in range(8):
                nc.sync.dma_start(out=wv_sb[:, 2 * g:2 * g + 2, :],
                                  in_=wr_v[:, 2 * g:2 * g + 2, :])

            def v_piece_dma(p):
                x_sb = xvp.tile([128, NHC, VC], bf16, tag="xv", name=f"xv{p}")
                xr = xT[:, p * VC:(p + 1) * VC].rearrange(
                    "(a p) m -> p a m", p=128)
                for g in range(4):
                    nc.sync.dma_start(out=x_sb[:, 4 * g:4 * g + 4, :],
                                      in_=xr[:, 4 * g:4 * g + 4, :])
                return x_sb

            def v_pb_mms(p, pb, x_sb):
                # serial over dc: one aux PSUM bank held at a time
                for dc in range(2):
                    psd = aux.tile([128, QC], fp32, tag="aux",
                                   name=f"psv{p}_{pb}_{dc}")
                    for hc in range(NHC):
                        nc.tensor.matmul(
                            psd[:],
                            x_sb[:, hc, pb * 128:(pb + 1) * 128],
                            wv_sb[:, hc, dc * QC:(dc + 1) * QC],
                            start=(hc == 0), stop=(hc == NHC - 1))
                    nc.scalar.copy(
                        Va[:, p * (VC // 128) + pb, dc * QC:(dc + 1) * QC],
                        psd[:])

            # V pieces 0-5 up front (jpair (0,1) needs pos < 1024);
            # pieces 6,7 are PE filler inside the ACT-bound jpair (0,1)
            for p in range(6):
                xv = v_piece_dma(p)
                for pb in range(VC // 128):
                    v_pb_mms(p, pb, xv)

            # ---------------- phase B: attention ----------------
            def attn_head(h, jpair):
                last_s = [None]
                j0 = jpair[0]
                blocks = {}
                first_i = {}
                last_i = {}
                for i in range(NKB):
                    grp = []
                    for j in jpair:
                        live = [t for t in range(4) if kind[4 * j + t][i] != 0]
                        if live:
                            grp.append((j, live[0] * 128))
                            if j not in first_i:
                                first_i[j] = i
                            last_i[j] = i
                    if grp:
                        blocks[i] = grp
                if not first_i:
                    return
                pso = {j: ps_o.tile([128, QC], fp32, tag=f"pso{j % 2}",
                                    name=f"pso{h}_{j}")
                       for j in first_i}
                # one bf16 pair accumulator [q-cols of both chunks]
                pacc = accp.tile([128, 2 * QC], bf16, tag="pa",
                                 name=f"pa{h}_{j0}")

                def emit_s(i, grp):
                    pss = ps_s.tile([128, 2 * QC], fp32, tag="pss",
                                    name=f"pss{h}_{i}")
                    for j, w0 in grp:
                        off = (j - j0) * QC
                        last_s[0] = nc.tensor.matmul(
                            pss[:, off + w0:off + QC],
                            KTa[:, h, i * 128:(i + 1) * 128],
                            QTa[:, h, j * QC + w0:(j + 1) * QC],
                            start=True, stop=True)
                    P = ppool.tile([128, 2 * QC], bf16, tag="p",
                                   name=f"p{h}_{i}")
                    s0 = (grp[0][0] - j0) * QC + grp[0][1]
                    nc.scalar.activation(P[:, s0:2 * QC], pss[:, s0:2 * QC],
                                         EXP, scale=SCALE)
                    for j, w0 in grp:
                        off = (j - j0) * QC
                        for t in range(w0 // 128, 4):
                            qb = 4 * j + t
                            if kind[qb][i] == 2:
                                pat = block_pat[(qb, i)]
                                nc.vector.tensor_mul(
                                    P[:, off + t * 128:off + (t + 1) * 128],
                                    P[:, off + t * 128:off + (t + 1) * 128],
                                    cst[:, pat * 128:(pat + 1) * 128])
                    # denominator accumulation: the union range is contiguous
                    if i == 0:
                        nc.vector.tensor_copy(pacc[:], P[:])
                    else:
                        nc.vector.tensor_add(pacc[:, s0:2 * QC],
                                             pacc[:, s0:2 * QC],
                                             P[:, s0:2 * QC])
                    return (i, grp, P)

                def emit_ovr(i, grp, P):
                    for j, w0 in grp:
                        off = (j - j0) * QC
                        nc.tensor.matmul(
                            pso[j][:, w0:QC],
                            Va[:, i, h * HD:(h + 1) * HD],
                            P[:, off + w0:off + QC],
                            start=(first_i[j] == i), stop=(last_i[j] == i))
                    for j, w0 in grp:
                        if last_i[j] != i:
                            continue
                        off = (j - j0) * QC
                        psr = aux.tile([128, QC], fp32, tag="aux",
                                       name=f"psr{h}_{j}")
                        nc.tensor.matmul(psr[0:1, :], ones_sb,
                                         pacc[:, off:off + QC],
                                         start=True, stop=True)
                        rc = redp.tile([1, QC], fp32, tag="rc",
                                       name=f"rc{h}_{j}")
                        nc.vector.reciprocal_approx_fast(out=rc[:],
                                                         in_=psr[0:1, :])
                        rb = redp.tile([1, QC], bf16, tag="rb",
                                       name=f"rb{h}_{j}")
                        nc.vector.tensor_copy(rb[:], rc[:])
                        bc = redp.tile([128, QC], bf16, tag="bc",
                                       name=f"bc{h}_{j}")
                        nc.gpsimd.partition_broadcast(bc[:], rb[:])
                        nc.vector.tensor_mul(
                            OTa[:, h, j * QC:(j + 1) * QC], pso[j][:], bc[:])

                prev = None
                for i in sorted(blocks):
                    cur = emit_s(i, blocks[i])
                    if prev is not None:
                        emit_ovr(*prev)
                    prev = cur
                if prev is not None:
                    emit_ovr(*prev)
                return last_s[0]

            # jpair (0,1): ACT-bound; fill the PE with V pieces 6,7
            xv_tiles = {6: v_piece_dma(6), 7: v_piece_dma(7)}
            for h in range(HPC):
                attn_head(h, (0, 1))
                if h % 2 == 1:
                    p, pb = 6 + h // 4, (h // 2) % 2
                    v_pb_mms(p, pb, xv_tiles[p])

            xvp_cm.__exit__(None, None, None)
            wvp_cm.__exit__(None, None, None)

            # Wo weights, streamed in 4 column pieces on the act ring
            wop_cm = tc.tile_pool(name="wop", bufs=1, side="right")
            wop = wop_cm.__enter__()
            ysb_cm = tc.tile_pool(name="ysb", bufs=2, side="right")
            ysb = ysb_cm.__enter__()
            wo_sb = wop.tile([128, HPC, H], bf16, tag="wo")
            wr_o = woT[:].rearrange("(a p) m -> p a m", p=128)
            for g in range(8):
                gs = slice(g * 256, (g + 1) * 256)
                nc.sync.dma_start(out=wo_sb[:, :, gs], in_=wr_o[:, :, gs])

            wo_alt = [0]

            def wo_block(jc, oc, dep=None):
                from concourse.tile import add_dep_helper
                ps = aux.tile([128, QC], fp32, tag="aux",
                              name=f"psc{jc}_{oc}")
                for fc in range(HPC):
                    mm = nc.tensor.matmul(
                        ps[:],
                        wo_sb[:, fc, oc * 128:(oc + 1) * 128],
                        OTa[:, fc, jc * QC:(jc + 1) * QC],
                        start=(fc == 0), stop=(fc == HPC - 1))
                    if fc == 0 and dep is not None:
                        add_dep_helper(mm.ins, dep.ins,
                                       reason="keep filler in head slot")
                y_sb = ysb.tile([128, QC], fp32, tag="y",
                                name=f"y{jc}_{oc}")
                if wo_alt[0] % 2:
                    nc.scalar.copy(y_sb[:], ps[:])
                    nc.scalar.dma_start(
                        out=yT[oc * 128:(oc + 1) * 128, jc * QC:(jc + 1) * QC],
                        in_=y_sb[:])
                else:
                    nc.vector.tensor_copy(y_sb[:], ps[:])
                    nc.sync.dma_start(
                        out=yT[oc * 128:(oc + 1) * 128, jc * QC:(jc + 1) * QC],
                        in_=y_sb[:])
                wo_alt[0] += 1

            wo_q = [(jc, oc) for jc in (0, 1) for oc in range(H // 128)]

            # jpair (2,3): fill the PE with Wo chunks 0/1
            for h in range(HPC):
                dep = attn_head(h, (2, 3))
                for _ in range(3):
                    if wo_q:
                        jc, oc = wo_q.pop(0)
                        wo_block(jc, oc, dep)

            # ---------------- phase C: remaining output projection --------
            wo_q += [(jc, oc) for jc in (2, 3) for oc in range(H // 128)]
            for jc, oc in wo_q:
                wo_block(jc, oc)

            ysb_cm.__exit__(None, None, None)
            wop_cm.__exit__(None, None, None)
            redp_cm.__exit__(None, None, None)
            accp_cm.__exit__(None, None, None)
            ppool_cm.__exit__(None, None, None)
            aux_cm.__exit__(None, None, None)
            ps_o_cm.__exit__(None, None, None)
            ps_s_cm.__exit__(None, None, None)
            otp_cm.__exit__(None, None, None)
            vp_cm.__exit__(None, None, None)

    nc.compile()
    return nc


def _prep_inputs(x, mask, Wq, Wk, Wv, Wo, patterns):
    import ml_dtypes
    bf16 = ml_dtypes.bfloat16

    # RoPE tables, d-major [HD, L]
    inv_freq = 1.0 / (ROPE_BASE ** (np.arange(0, HD, 2, dtype=np.float64)
                                    / HD))
    t = np.arange(L, dtype=np.float64)
    freqs = np.outer(t, inv_freq)                     # [L, HD/2]
    emb = np.concatenate((freqs, freqs), axis=-1)     # [L, HD]
    cos = np.cos(emb).T.astype(np.float32)            # [HD, L]
    sin = np.sin(emb).T.astype(np.float32)
    sinm = sin.copy()
    sinm[0:64] = -sin[0:64]
    cos_b = cos.astype(bf16)
    sinm_b = sinm.astype(bf16)

    npat = max(len(patterns), 1)
    maskd = np.zeros((npat, 128, 128), dtype=bf16)
    for i, p in enumerate(patterns):
        maskd[i] = p.astype(np.float32).astype(bf16)

    in_maps = []
    for c in range(NCORES):
        b, half = c // 2, c % 2
        rows = slice(half * HPC * HD, (half + 1) * HPC * HD)
        in_maps.append({
            "xT": np.ascontiguousarray(x[b].T).astype(bf16),
            "wqT": np.ascontiguousarray(Wq[rows, :].T).astype(bf16),
            "wkT": np.ascontiguousarray(Wk[rows, :].T).astype(bf16),
            "wvT": np.ascontiguousarray(Wv[rows, :].T).astype(bf16),
            "woT": np.ascontiguousarray(Wo[:, rows].T).astype(bf16),
            "cosd": cos_b,
            "sinmd": sinm_b,
            "maskd": maskd,
        })
    return in_maps


def kernel(x, mask, Wq, Wk, Wv, Wo, _trace=False):
    from concourse.bass_utils import run_bass_kernel_spmd

    x = np.asarray(x, dtype=np.float32)
    mask2d = np.asarray(mask, dtype=np.int32).reshape(L, L)
    key = mask2d.tobytes()
    if key not in _cache:
        kind, patterns, block_pat = _analyze_mask(mask2d)
        nc = _build(kind, block_pat, len(patterns))
        _cache[key] = (nc, patterns)
    nc, patterns = _cache[key]

    in_maps = _prep_inputs(x, mask, np.asarray(Wq, np.float32),
                           np.asarray(Wk, np.float32),
                           np.asarray(Wv, np.float32),
                           np.asarray(Wo, np.float32), patterns)
    res = run_bass_kernel_spmd(nc, in_maps, list(range(NCORES)),
                               trace=_trace)
    y = np.empty((B, L, H), dtype=np.float32)
    for b in range(B):
        acc = res.results[2 * b]["yT"].astype(np.float32) + \
              res.results[2 * b + 1]["yT"].astype(np.float32)
        y[b] = acc.T
    if _trace:
        kernel.last_results = res
    return y


if __name__ == "__main__":
    import reference
    inputs = reference.setup_inputs()
    inputs = {k: np.asarray(v) for k, v in inputs.items()}
    out = kernel(**inputs)
    exp = np.asarray(reference.reference(**{k: v for k, v in inputs.items()}))
    err = np.abs(out - exp).max() / np.abs(exp).max()
    print("rel err (absmax):", err)


# revision 19
# speedup vs baseline: 1.1793x; 1.1793x over previous
"""Trainium2 Bass kernel: causal multi-head attention with RoPE.

Model: B=4, L=2048, H=2048, NH=16 heads, head_dim=128.
  q = x @ Wq.T ; k = x @ Wk.T ; v = x @ Wv.T        (per-head split)
  q, k <- RoPE(q, k)
  attn = softmax(mask(q k^T / sqrt(hd)))
  out  = (attn @ v) heads-concat @ Wo.T

Sharding (8 cores): hybrid batch x tensor-parallel.  Core c handles
batch b = c//2 and heads half*8..half*8+7 with half = c%2.  Wq/Wk/Wv are
column-sharded (8 heads per core), Wo row-sharded; each core produces a
partial y[b] and the host sums the two partials per batch (the unshard
step) and concatenates batches.

Per-core dataflow (SBUF-resident, bf16 inputs / fp32 accumulation):
  phase A: merged Q^T/K^T projection per 512-pos x chunk (x loaded once
           for both) with fused RoPE; startup DMA split in small pieces
           across both HWDGE rings; gpsimd ucode lib pre-warmed.
  phase B: flash-style causal attention per (head, 512-q-chunk PAIR):
           the two chunks share one 2-bank PSUM S^T tile so ONE exp
           ACTIVATE covers both (the 352-cycle ACT fixed cost was
           co-critical with the PE); softmax denominators accumulate on
           DVE in fp16 and partition-reduce on gpsimd (partition_all_
           reduce) -- no PE ones-matmuls.  V pieces 4-7 and the Wo
           chunks 0/1 are emitted as PE filler inside the ACT-bound
           attention stretches.
  phase C: remaining output projection, y^T -> DRAM fp32.
"""

import math
import numpy as np

B, L, H, NH, HD = 4, 2048, 2048, 16, 128
ROPE_BASE = 10000.0
NCORES = 8
HPC = 8          # heads per core
QC = 512         # q chunk width
NQC = L // QC    # 4 q chunks
NKB = L // 128   # 16 kp blocks
VC = 256         # V-phase x piece width (pos)
SCALE = 1.0 / math.sqrt(HD)

_cache = {}


def _analyze_mask(mask2d):
    """Classify each (q_block, kp_block) 128x128 block of the [L, L] mask.

    Returns (block_kind[16][16] with 0=empty,1=full,2=mixed, patterns,
    pattern_idx dict keyed by block coords). mask2d is int32 [L, L],
    rows=q, cols=kp.
    """
    nb = L // 128
    kind = [[0] * nb for _ in range(nb)]
    patterns = []
    pat_key_to_idx = {}
    block_pat = {}
    for qb in range(nb):
        rows = mask2d[qb * 128:(qb + 1) * 128]
        for kb in range(nb):
            blk = rows[:, kb * 128:(kb + 1) * 128]
            s = int(blk.sum())
            if s == 0:
                kind[qb][kb] = 0
            elif s == 128 * 128:
                kind[qb][kb] = 1
            else:
                kind[qb][kb] = 2
                key = blk.tobytes()
                idx = pat_key_to_idx.get(key)
                if idx is None:
                    idx = len(patterns)
                    pat_key_to_idx[key] = idx
                    # stored transposed: S^T tiles are [kp, q]
                    patterns.append(np.ascontiguousarray(blk.T))
                block_pat[(qb, kb)] = idx
    return kind, patterns, block_pat


def _build(kind, block_pat, n_patterns):
    """Build the SPMD bass program (same for all 8 cores)."""
    import concourse.bass as bass
    import concourse.bass_isa as bass_isa
    import concourse.bacc as bacc
    import concourse.mybir as mybir
    import concourse.tile as tile

    fp32 = mybir.dt.float32
    bf16 = mybir.dt.bfloat16
    EXP = mybir.ActivationFunctionType.Exp

    nc = bacc.Bacc("TRN2", target_bir_lowering=False, debug=False)

    xT = nc.dram_tensor("xT", [H, L], bf16, kind="ExternalInput")
    wqT = nc.dram_tensor("wqT", [H, HPC * HD], bf16, kind="ExternalInput")
    wkT = nc.dram_tensor("wkT", [H, HPC * HD], bf16, kind="ExternalInput")
    wvT = nc.dram_tensor("wvT", [H, HPC * HD], bf16, kind="ExternalInput")
    woT = nc.dram_tensor("woT", [HPC * HD, H], bf16, kind="ExternalInput")
    cosd = nc.dram_tensor("cosd", [HD, L], bf16, kind="ExternalInput")
    sinmd = nc.dram_tensor("sinmd", [HD, L], bf16, kind="ExternalInput")
    npat = max(n_patterns, 1)
    maskd = nc.dram_tensor("maskd", [npat, 128, 128], bf16, kind="ExternalInput")
    yT = nc.dram_tensor("yT", [H, L], fp32, kind="ExternalOutput")

    NHC = H // 128  # 16 input-feature blocks

    with tile.TileContext(nc) as tc:
        with tc.tile_pool(name="persist", bufs=1, side="left") as persist:
            cst = persist.tile([128, npat * 128 + 128], bf16, tag="cst")
            warm = persist.tile([128, 8], fp32, tag="warm")
            QTa = persist.tile([HD, HPC, L], bf16, tag="qta")
            KTa = persist.tile([HD, HPC, L], bf16, tag="kta")

            # pre-load the gpsimd "attn" ucode library during startup DMA
            # (the first custom op otherwise pays a ~10us LOAD_LIB mid-kernel)
            nc.vector.memset(warm[:], 1.0)
            nc.gpsimd.partition_broadcast(warm[:, 4:8], warm[0:1, 0:4])

            # wv low half: staged in a pool that outlives phase A, so its
            # DMA has no anti-dependency on phase-A SBUF and V-projection
            # matmuls can start the moment phase A drains
            wvs_cm = tc.tile_pool(name="wvs", bufs=1, side="right")
            wvs = wvs_cm.__enter__()
            wv_lo = wvs.tile([128, NHC // 2, HPC * HD], bf16, tag="wvlo")
            wr_v = wvT[:].rearrange("(a p) m -> p a m", p=128)

            # ---------------- phase A: Q/K projections + RoPE -------------
            wqk_cm = tc.tile_pool(name="wqk", bufs=1, side="right")
            wqk = wqk_cm.__enter__()
            ropec_cm = tc.tile_pool(name="ropec", bufs=1, side="right")
            ropec = ropec_cm.__enter__()
            xqk_cm = tc.tile_pool(name="xqk", bufs=2, side="right")
            xqk = xqk_cm.__enter__()
            tpool_cm = tc.tile_pool(name="tpool", bufs=2, side="right")
            tpool = tpool_cm.__enter__()
            psp_cm = tc.tile_pool(name="ps_proj", bufs=3, space="PSUM")
            psp = psp_cm.__enter__()

            wq_sb = wqk.tile([128, NHC, HPC * HD], bf16, tag="wq")
            wk_sb = wqk.tile([128, NHC, HPC * HD], bf16, tag="wk")
            cos_sb = ropec.tile([HD, L], bf16, tag="cos")
            sinm_sb = ropec.tile([HD, L], bf16, tag="sinm")

            wr_q = wqT[:].rearrange("(a p) m -> p a m", p=128)
            wr_k = wkT[:].rearrange("(a p) m -> p a m", p=128)

            def x_tile_dma(x_sb, js):
                xr = xT[:, js].rearrange("(a p) m -> p a m", p=128)
                for g in range(8):
                    nc.sync.dma_start(out=x_sb[:, 2 * g:2 * g + 2, :],
                                      in_=xr[:, 2 * g:2 * g + 2, :])

            # startup: interleave wq (sync ring) with x0 (act ring) in
            # small pieces so the first matmuls can start early
            x0_sb = xqk.tile([128, NHC, QC], bf16, tag="x", name="x0")
            xr0 = xT[:, 0:QC].rearrange("(a p) m -> p a m", p=128)
            for g in range(8):
                nc.sync.dma_start(out=wq_sb[:, 2 * g:2 * g + 2, :],
                                  in_=wr_q[:, 2 * g:2 * g + 2, :])
                nc.scalar.dma_start(out=x0_sb[:, 2 * g:2 * g + 2, :],
                                    in_=xr0[:, 2 * g:2 * g + 2, :])
            for g in range(8):
                nc.sync.dma_start(out=wk_sb[:, 2 * g:2 * g + 2, :],
                                  in_=wr_k[:, 2 * g:2 * g + 2, :])
            for g in range(4):
                nc.sync.dma_start(out=wv_lo[:, 2 * g:2 * g + 2, :],
                                  in_=wr_v[:, 2 * g:2 * g + 2, :])
            nc.scalar.dma_start(out=cos_sb[:], in_=cosd[:])
            nc.scalar.dma_start(out=sinm_sb[:], in_=sinmd[:])
            for p in range(n_patterns):
                nc.scalar.dma_start(out=cst[:, p * 128:(p + 1) * 128],
                                    in_=maskd[p])
            ones_col = npat * 128
            nc.vector.memset(cst[:, ones_col:ones_col + 128], 1.0)
            ones_sb = cst[:, ones_col:ones_col + 1]

            for j in range(NQC):
                js = slice(j * QC, (j + 1) * QC)
                if j == 0:
                    x_sb = x0_sb
                else:
                    x_sb = xqk.tile([128, NHC, QC], bf16, tag="x",
                                    name=f"x{j}")
                    x_tile_dma(x_sb, js)
                for w_sb, out_a, wtag in ((wq_sb, QTa, "q"), (wk_sb, KTa, "k")):
                    for h in range(HPC):
                        ps = psp.tile([128, QC], fp32, tag="ps_proj")
                        for hc in range(NHC):
                            nc.tensor.matmul(
                                ps[:],
                                w_sb[:, hc, h * HD:(h + 1) * HD],
                                x_sb[:, hc, :],
                                start=(hc == 0), stop=(hc == NHC - 1))
                        q = out_a[:, h, js]
                        nc.scalar.copy(q, ps[:])
                        # rotate-half: pure partition swap via SBUF->SBUF DMA
                        # (scalar ring: keeps the sync ring a pure prefetch
                        # stream with no compute-dependent head-of-line waits)
                        rq = tpool.tile([128, QC], bf16, tag="rot")
                        nc.scalar.dma_start(out=rq[0:64, :],
                                            in_=out_a[64:128, h, js])
                        nc.scalar.dma_start(out=rq[64:128, :],
                                            in_=out_a[0:64, h, js])
                        nc.vector.tensor_mul(rq[:], rq[:], sinm_sb[:, js])
                        nc.vector.tensor_mul(q, q, cos_sb[:, js])
                        nc.vector.tensor_add(q, q, rq[:])

            tpool_cm.__exit__(None, None, None)
            xqk_cm.__exit__(None, None, None)
            ropec_cm.__exit__(None, None, None)
            wqk_cm.__exit__(None, None, None)
            psp_cm.__exit__(None, None, None)

            # ---------------- V projection + attention + output -----------
            vp_cm = tc.tile_pool(name="vp", bufs=1, side="left")
            vp = vp_cm.__enter__()
            Va = vp.tile([128, NKB, HPC * HD], bf16, tag="va")
            otp_cm = tc.tile_pool(name="otp", bufs=1, side="left")
            otp = otp_cm.__enter__()
            OTa = otp.tile([HD, HPC, L], bf16, tag="ota")

            ppool_cm = tc.tile_pool(name="pp", bufs=3, side="right")
            ppool = ppool_cm.__enter__()
            accp_cm = tc.tile_pool(name="acc", bufs=2, side="right")
            accp = accp_cm.__enter__()
            redp_cm = tc.tile_pool(name="red", bufs=1, side="right")
            redp = redp_cm.__enter__()
            wvp_cm = tc.tile_pool(name="wvp", bufs=1, side="right")
            wvp = wvp_cm.__enter__()
            xvp_cm = tc.tile_pool(name="xvp", bufs=3, side="right")
            xvp = xvp_cm.__enter__()

            ps_s_cm = tc.tile_pool(name="ps_s", bufs=2, space="PSUM")
            ps_s = ps_s_cm.__enter__()
            ps_o_cm = tc.tile_pool(name="ps_o", bufs=1, space="PSUM")
            ps_o = ps_o_cm.__enter__()
            aux_cm = tc.tile_pool(name="ps_aux", bufs=2, space="PSUM")
            aux = aux_cm.__enter__()

            wv_hi = wvp.tile([128, NHC // 2, HPC * HD], bf16, tag="wvhi")
            for g in range(4):
                nc.sync.dma_start(out=wv_hi[:, 2 * g:2 * g + 2, :],
                                  in_=wr_v[:, 8 + 2 * g:8 + 2 * g + 2, :])

            def v_piece_dma(p):
                x_sb = xvp.tile([128, NHC, VC], bf16, tag="xv", name=f"xv{p}")
                xr = xT[:, p * VC:(p + 1) * VC].rearrange(
                    "(a p) m -> p a m", p=128)
                for g in range(4):
                    nc.sync.dma_start(out=x_sb[:, 4 * g:4 * g + 4, :],
                                      in_=xr[:, 4 * g:4 * g + 4, :])
                return x_sb

            def v_pb_mms(p, pb, x_sb):
                # serial over dc: one aux PSUM bank held at a time
                for dc in range(2):
                    psd = aux.tile([128, QC], fp32, tag="aux",
                                   name=f"psv{p}_{pb}_{dc}")
                    for hc in range(NHC):
                        w = (wv_lo[:, hc, dc * QC:(dc + 1) * QC] if hc < 8
                             else wv_hi[:, hc - 8, dc * QC:(dc + 1) * QC])
                        nc.tensor.matmul(
                            psd[:],
                            x_sb[:, hc, pb * 128:(pb + 1) * 128],
                            w,
                            start=(hc == 0), stop=(hc == NHC - 1))
                    nc.scalar.copy(
                        Va[:, p * (VC // 128) + pb, dc * QC:(dc + 1) * QC],
                        psd[:])

            # V pieces 0-5 up front (jpair (0,1) needs pos < 1024);
            # pieces 6,7 are PE filler inside the ACT-bound jpair (0,1)
            for p in range(6):
                xv = v_piece_dma(p)
                for pb in range(VC // 128):
                    v_pb_mms(p, pb, xv)

            # ---------------- phase B: attention ----------------
            def attn_head(h, jpair):
                last_s = [None]
                j0 = jpair[0]
                blocks = {}
                first_i = {}
                last_i = {}
                for i in range(NKB):
                    grp = []
                    for j in jpair:
                        live = [t for t in range(4) if kind[4 * j + t][i] != 0]
                        if live:
                            grp.append((j, live[0] * 128))
                            if j not in first_i:
                                first_i[j] = i
                            last_i[j] = i
                    if grp:
                        blocks[i] = grp
                if not first_i:
                    return
                pso = {j: ps_o.tile([128, QC], fp32, tag=f"pso{j % 2}",
                                    name=f"pso{h}_{j}")
                       for j in first_i}
                # one bf16 pair accumulator [q-cols of both chunks]
                pacc = accp.tile([128, 2 * QC], bf16, tag="pa",
                                 name=f"pa{h}_{j0}")

                def emit_s(i, grp):
                    pss = ps_s.tile([128, 2 * QC], fp32, tag="pss",
                                    name=f"pss{h}_{i}")
                    for j, w0 in grp:
                        off = (j - j0) * QC
                        nc.tensor.matmul(
                            pss[:, off + w0:off + QC],
                            KTa[:, h, i * 128:(i + 1) * 128],
                            QTa[:, h, j * QC + w0:(j + 1) * QC],
                            start=True, stop=True)
                    P = ppool.tile([128, 2 * QC], bf16, tag="p",
                                   name=f"p{h}_{i}")
                    s0 = (grp[0][0] - j0) * QC + grp[0][1]
                    last_s[0] = nc.scalar.activation(
                        P[:, s0:2 * QC], pss[:, s0:2 * QC], EXP, scale=SCALE)
                    for j, w0 in grp:
                        off = (j - j0) * QC
                        for t in range(w0 // 128, 4):
                            qb = 4 * j + t
                            if kind[qb][i] == 2:
                                pat = block_pat[(qb, i)]
                                nc.vector.tensor_mul(
                                    P[:, off + t * 128:off + (t + 1) * 128],
                                    P[:, off + t * 128:off + (t + 1) * 128],
                                    cst[:, pat * 128:(pat + 1) * 128])
                    # denominator accumulation: the union range is contiguous
                    if i == 0:
                        nc.vector.tensor_copy(pacc[:], P[:])
                    else:
                        nc.vector.tensor_add(pacc[:, s0:2 * QC],
                                             pacc[:, s0:2 * QC],
                                             P[:, s0:2 * QC])
                    return (i, grp, P)

                def emit_ovr(i, grp, P):
                    for j, w0 in grp:
                        off = (j - j0) * QC
                        nc.tensor.matmul(
                            pso[j][:, w0:QC],
                            Va[:, i, h * HD:(h + 1) * HD],
                            P[:, off + w0:off + QC],
                            start=(first_i[j] == i), stop=(last_i[j] == i))
                    for j, w0 in grp:
                        if last_i[j] != i:
                            continue
                        off = (j - j0) * QC
                        psr = aux.tile([128, QC], fp32, tag="aux",
                                       name=f"psr{h}_{j}")
                        nc.tensor.matmul(psr[0:1, :], ones_sb,
                                         pacc[:, off:off + QC],
                                         start=True, stop=True)
                        rc = redp.tile([1, QC], fp32, tag="rc",
                                       name=f"rc{h}_{j}")
                        nc.vector.reciprocal_approx_fast(out=rc[:],
                                                         in_=psr[0:1, :])
                        rb = redp.tile([1, QC], bf16, tag="rb",
                                       name=f"rb{h}_{j}")
                        nc.vector.tensor_copy(rb[:], rc[:])
                        bc = redp.tile([128, QC], bf16, tag="bc",
                                       name=f"bc{h}_{j}")
                        nc.gpsimd.partition_broadcast(bc[:], rb[:])
                        nc.vector.tensor_mul(
                            OTa[:, h, j * QC:(j + 1) * QC], pso[j][:], bc[:])

                prev = None
                for i in sorted(blocks):
                    cur = emit_s(i, blocks[i])
                    if prev is not None:
                        emit_ovr(*prev)
                    prev = cur
                if prev is not None:
                    emit_ovr(*prev)
                return last_s[0]

            # jpair (0,1): ACT-bound; fill the PE with V pieces 6,7
            xv_tiles = {6: v_piece_dma(6), 7: v_piece_dma(7)}
            for h in range(HPC):
                attn_head(h, (0, 1))
                if h % 2 == 1:
                    p, pb = 6 + h // 4, (h // 2) % 2
                    v_pb_mms(p, pb, xv_tiles[p])

            xvp_cm.__exit__(None, None, None)
            wvp_cm.__exit__(None, None, None)

            # Wo weights, streamed in 4 column pieces on the act ring
            wop_cm = tc.tile_pool(name="wop", bufs=1, side="right")
            wop = wop_cm.__enter__()
            ysb_cm = tc.tile_pool(name="ysb", bufs=2, side="right")
            ysb = ysb_cm.__enter__()
            wo_sb = wop.tile([128, HPC, H], bf16, tag="wo")
            wr_o = woT[:].rearrange("(a p) m -> p a m", p=128)
            for g in range(8):
                gs = slice(g * 256, (g + 1) * 256)
                nc.sync.dma_start(out=wo_sb[:, :, gs], in_=wr_o[:, :, gs])

            wo_alt = [0]

            def wo_block(jc, oc, dep=None):
                from concourse.tile import add_dep_helper
                ps = aux.tile([128, QC], fp32, tag="aux",
                              name=f"psc{jc}_{oc}")
                for fc in range(HPC):
                    mm = nc.tensor.matmul(
                        ps[:],
                        wo_sb[:, fc, oc * 128:(oc + 1) * 128],
                        OTa[:, fc, jc * QC:(jc + 1) * QC],
                        start=(fc == 0), stop=(fc == HPC - 1))
                    if fc == 0 and dep is not None:
                        add_dep_helper(mm.ins, dep.ins,
                                       reason="keep filler in head slot")
                y_sb = ysb.tile([128, QC], fp32, tag="y",
                                name=f"y{jc}_{oc}")
                if wo_alt[0] % 2:
                    nc.scalar.copy(y_sb[:], ps[:])
                    nc.scalar.dma_start(
                        out=yT[oc * 128:(oc + 1) * 128, jc * QC:(jc + 1) * QC],
                        in_=y_sb[:])
                else:
                    nc.vector.tensor_copy(y_sb[:], ps[:])
                    nc.sync.dma_start(
                        out=yT[oc * 128:(oc + 1) * 128, jc * QC:(jc + 1) * QC],
                        in_=y_sb[:])
                wo_alt[0] += 1

            wo_q = [(jc, oc) for jc in (0, 1) for oc in range(H // 128)]

            # jpair (2,3): fill the PE with Wo chunks 0/1
            for h in range(HPC):
                dep = attn_head(h, (2, 3))
                for k in range(3):
                    if wo_q:
                        jc, oc = wo_q.pop(0)
                        wo_block(jc, oc, dep if k < 2 else None)

            # ---------------- phase C: remaining output projection --------
            wo_q += [(jc, oc) for jc in (2, 3) for oc in range(H // 128)]
            for jc, oc in wo_q:
                wo_block(jc, oc)

            ysb_cm.__exit__(None, None, None)
            wop_cm.__exit__(None, None, None)
            redp_cm.__exit__(None, None, None)
            accp_cm.__exit__(None, None, None)
            ppool_cm.__exit__(None, None, None)
            aux_cm.__exit__(None, None, None)
            ps_o_cm.__exit__(None, None, None)
            ps_s_cm.__exit__(None, None, None)
            wvs_cm.__exit__(None, None, None)
            otp_cm.__exit__(None, None, None)
            vp_cm.__exit__(None, None, None)

    nc.compile()
    return nc


def _prep_inputs(x, mask, Wq, Wk, Wv, Wo, patterns):
    import ml_dtypes
    bf16 = ml_dtypes.bfloat16

    # RoPE tables, d-major [HD, L]
    inv_freq = 1.0 / (ROPE_BASE ** (np.arange(0, HD, 2, dtype=np.float64)
                                    / HD))
    t = np.arange(L, dtype=np.float64)
    freqs = np.outer(t, inv_freq)                     # [L, HD/2]
    emb = np.concatenate((freqs, freqs), axis=-1)     # [L, HD]
    cos = np.cos(emb).T.astype(np.float32)            # [HD, L]
    sin = np.sin(emb).T.astype(np.float32)
    sinm = sin.copy()
    sinm[0:64] = -sin[0:64]
    cos_b = cos.astype(bf16)
    sinm_b = sinm.astype(bf16)

    npat = max(len(patterns), 1)
    maskd = np.zeros((npat, 128, 128), dtype=bf16)
    for i, p in enumerate(patterns):
        maskd[i] = p.astype(np.float32).astype(bf16)

    in_maps = []
    for c in range(NCORES):
        b, half = c // 2, c % 2
        rows = slice(half * HPC * HD, (half + 1) * HPC * HD)
        in_maps.append({
            "xT": np.ascontiguousarray(x[b].T).astype(bf16),
            "wqT": np.ascontiguousarray(Wq[rows, :].T).astype(bf16),
            "wkT": np.ascontiguousarray(Wk[rows, :].T).astype(bf16),
            "wvT": np.ascontiguousarray(Wv[rows, :].T).astype(bf16),
            "woT": np.ascontiguousarray(Wo[:, rows].T).astype(bf16),
            "cosd": cos_b,
            "sinmd": sinm_b,
            "maskd": maskd,
        })
    return in_maps


def kernel(x, mask, Wq, Wk, Wv, Wo, _trace=False):
    from concourse.bass_utils import run_bass_kernel_spmd

    x = np.asarray(x, dtype=np.float32)
    mask2d = np.asarray(mask, dtype=np.int32).reshape(L, L)
    key = mask2d.tobytes()
    if key not in _cache:
        kind, patterns, block_pat = _analyze_mask(mask2d)
        nc = _build(kind, block_pat, len(patterns))
        _cache[key] = (nc, patterns)
    nc, patterns = _cache[key]

    in_maps = _prep_inputs(x, mask, np.asarray(Wq, np.float32),
                           np.asarray(Wk, np.float32),
                           np.asarray(Wv, np.float32),
                           np.asarray(Wo, np.float32), patterns)
    res = run_bass_kernel_spmd(nc, in_maps, list(range(NCORES)),
                               trace=_trace)
    y = np.empty((B, L, H), dtype=np.float32)
    for b in range(B):
        acc = res.results[2 * b]["yT"].astype(np.float32) + \
              res.results[2 * b + 1]["yT"].astype(np.float32)
        y[b] = acc.T
    if _trace:
        kernel.last_results = res
    return y


if __name__ == "__main__":
    import reference
    inputs = reference.setup_inputs()
    inputs = {k: np.asarray(v) for k, v in inputs.items()}
    out = kernel(**inputs)
    exp = np.asarray(reference.reference(**{k: v for k, v in inputs.items()}))
    err = np.abs(out - exp).max() / np.abs(exp).max()
    print("rel err (absmax):", err)
